# revision 16
# baseline (speedup 1.0000x reference)
"""Multi-head attention Trainium2 kernel (nn_MultiHeadAttention_86423331930281).

Self-contained: builds a Bass/Tile SPMD kernel, data-parallel over batch
(B=8 -> one batch element per NeuronCore), runs on cores 0-7 via
run_bass_kernel_spmd, returns the full [8, 1024, 1024] output.

Host-side prep (per core): transpose q/k/v to [d, s] and cast bf16, pack
per-head-pair projection weights as [m, ki, ko, he], transpose Wo.  This
removes all PE transposes and staging copies from the device kernel.

Per-core algorithm (S=1024, D=1024, H=16, E=64):
  - Q/K-proj for head pair m -> QT/KT [he_pair=128, s] (bf16); m=0 runs
    first so PE starts ~2 DMAs in; proj for m+1 is emitted inside m's
    attention loop to hide the PSUM-evacuation latency.
  - V-proj:  V1[t, h, e|1] = vT.T @ Wv (bf16), trailing ones column per head
    (softmax denominators for free via the PV matmul)
  - attention per (m, j): scoresT[t,s] = KT_slice.T @ QT (K=64); exp with
    scale=1/32 folded, split ScalarE (spline exp) / DVE (bf16-bit-trick
    exp, whose common-mode bias cancels in softmax); PV accumulates
    attT[e|sum, s] over t in PSUM.
  - reciprocal of denominators, SBUF broadcast DMA, normalize on DVE,
    FC: out = attT.T @ WoT + bo
"""

import numpy as np
from contextlib import ExitStack

import concourse.bass as bass
import concourse.mybir as mybir
import concourse.tile as tile
from concourse.bass_utils import run_bass_kernel_spmd

P = 128
S = 1024          # sequence length
DK = 1024         # qkv input dim
H = 16            # heads
E = 64            # per-head dim
HE = H * E        # 1024
OUT = 1024        # output dim
NT = S // P       # 8 s/t tiles
NK = DK // P      # 8 contraction tiles
NM = H // 2       # 8 head pairs
F32 = mybir.dt.float32
F32R = mybir.dt.float32r
BF16 = mybir.dt.bfloat16
I16 = mybir.dt.int16
FP8 = mybir.dt.float8e4
AF = mybir.ActivationFunctionType
ALU = mybir.AluOpType
SCALE = 1.0 / 32.0  # 1/sqrt(DK)

# ---- tuning knobs -------------------------------------------------------
EXP_DVE_COLS = 256      # columns per (m,j,hh) exp tile handled by DVE
USE_PROJ_DR = False     # fp8 DoubleRow for Q/K projections
USE_SCORES_DR = False   # fp8 DoubleRow for score matmuls

# bf16-bit-trick exp on DVE: bits16 = round(x * EXPA + EXPB) reinterpreted
# as bf16 gives approx exp(x * SCALE).  The additive constant only shifts a
# common-mode factor which softmax normalization cancels exactly.
LOG2E = 1.4426950408889634
EXPA = SCALE * LOG2E * 128.0
EXPB = 16256.0 - 4.75


def _legalize_matmul_waits(nc):
    """This walrus build allows only ONE sync-wait command per Matmult.
    Move all but the last wait of any multi-wait matmul onto freshly
    inserted PE nops immediately before it — same engine queue, so the
    blocking semantics are identical."""
    SKIP = ("NoOp", "Br", "Halt", "Sem", "Event")
    k = 0
    for f in nc.m.functions:
        for b in f.blocks:
            out = []
            for inst in b.instructions:
                si = getattr(inst, "sync_info", None)
                tname = type(inst).__name__
                if (not any(s in tname for s in SKIP) and si is not None
                        and si.on_wait and len(si.on_wait) > 1):
                    waits = list(si.on_wait)
                    for w in waits[:-1]:
                        nop = mybir.InstNoOp(
                            name=f"legalize-nop-{k}", ins=[], outs=[])
                        k += 1
                        nop.engine = inst.engine
                        nop.sync_info = mybir.SyncInfo(
                            on_wait=[w], on_update=[])
                        out.append(nop)
                    inst.sync_info = mybir.SyncInfo(
                        on_wait=[waits[-1]], on_update=list(si.on_update))
                out.append(inst)
            b.instructions[:] = out
    return k


def build(legalize=True):
    nc = bass.Bass()
    qT_d = nc.dram_tensor("qT", (DK, S), BF16, kind="ExternalInput")
    kT_d = nc.dram_tensor("kT", (DK, S), BF16, kind="ExternalInput")
    vT_d = nc.dram_tensor("vT", (DK, S), BF16, kind="ExternalInput")
    # per head pair m: [ki, ko, he_pair]
    wq_d = nc.dram_tensor("wqp", (NM, P, NK, P), BF16, kind="ExternalInput")
    wk_d = nc.dram_tensor("wkp", (NM, P, NK, P), BF16, kind="ExternalInput")
    # [ko, ki, h*e]
    wv_d = nc.dram_tensor("wvp", (NK, P, HE), BF16, kind="ExternalInput")
    # Wo.T as [m, he_block, out]
    wo_d = nc.dram_tensor("woT", (NM, P, OUT), BF16, kind="ExternalInput")
    bo_d = nc.dram_tensor("bo", (OUT,), F32, kind="ExternalInput")
    out_d = nc.dram_tensor("out", (S, OUT), F32, kind="ExternalOutput")
    recip_d = nc.dram_tensor("recip_scratch", (H, S), BF16, kind="Internal")

    with tile.TileContext(nc) as tc, ExitStack() as ctx:
        const = ctx.enter_context(tc.tile_pool(name="const", bufs=1))
        xqk = ctx.enter_context(tc.tile_pool(name="xqk", bufs=NK))
        v1p = ctx.enter_context(tc.tile_pool(name="v1p", bufs=NT))
        woTp = ctx.enter_context(tc.tile_pool(name="woTp", bufs=NM))
        wslp = ctx.enter_context(tc.tile_pool(name="wslp", bufs=4))
        qkp = ctx.enter_context(tc.tile_pool(name="qkp", bufs=4))
        ptp = ctx.enter_context(tc.tile_pool(name="ptp", bufs=4))
        att65p = ctx.enter_context(tc.tile_pool(name="att65p", bufs=1))
        attp = ctx.enter_context(tc.tile_pool(name="attp", bufs=NM))
        sumsp = ctx.enter_context(tc.tile_pool(name="sumsp", bufs=1))
        rbcp = ctx.enter_context(tc.tile_pool(name="rbcp", bufs=4))
        outp = ctx.enter_context(tc.tile_pool(name="outp", bufs=3))
        ps = ctx.enter_context(tc.tile_pool(name="ps", bufs=2, space="PSUM"))
        ps_att = ctx.enter_context(
            tc.tile_pool(name="ps_att", bufs=2, space="PSUM"))
        ph1 = ExitStack()
        vwp = ph1.enter_context(tc.tile_pool(name="vwp", bufs=NK))

        ones_h = const.tile([P, H], BF16, name="ones_h")
        nc.gpsimd.memset(ones_h[:], 1.0)
        sums16 = [sumsp.tile([H // 2, S], BF16, name=f"sums16_{i}")
                  for i in range(2)]
        recip_bf = [sumsp.tile([H // 2, S], BF16, name=f"recip_bf{i}")
                    for i in range(2)]
        recip_f32 = [sumsp.tile([H // 2, S], F32, name=f"recip_f32{i}")
                     for i in range(2)]
        # all (m, hh) attT+denominator slots in one tile so the denominator
        # gather is a single DMA per batch
        att65 = att65p.tile([E + 1, H, S], BF16, name="att65")

        # ---- input DMAs.  SP queue: qT + wq/wk (+rbc later); Pool queue:
        # kT, vT, wv, woT, gathers, out stores.  proj m=0 starts ~2 DMAs in.
        qT_t, kT_t, vT_t, wv_t = [], [], [], []
        for j in range(NK):
            t = xqk.tile([P, S], BF16, name=f"qT{j}", tag="qT")
            nc.sync.dma_start(t[:], qT_d[j * P:(j + 1) * P, :])
            qT_t.append(t)
            t = xqk.tile([P, S], BF16, name=f"kT{j}", tag="kT")
            nc.gpsimd.dma_start(t[:], kT_d[j * P:(j + 1) * P, :])
            kT_t.append(t)

        def load_wqk(m, queue=None):
            q_ = queue or nc.sync
            wq = wslp.tile([P, NK, P], BF16, name=f"wq{m}", tag="wsl")
            wk = wslp.tile([P, NK, P], BF16, name=f"wk{m}", tag="wsl")
            q_.dma_start(wq[:], wq_d[m])
            q_.dma_start(wk[:], wk_d[m])
            return wq, wk

        wqk = [load_wqk(0), load_wqk(1)]

        for j in range(NK):
            t = vwp.tile([P, S], BF16, name=f"vT{j}", tag="vT")
            (nc.sync if j % 2 == 0 else nc.gpsimd).dma_start(
                t[:], vT_d[j * P:(j + 1) * P, :])
            vT_t.append(t)
        for j in range(NK):
            t = vwp.tile([P, HE], BF16, name=f"wv{j}", tag="wv")
            (nc.gpsimd if j % 2 == 0 else nc.sync).dma_start(t[:], wv_d[j])
            wv_t.append(t)

        bo_bc = const.tile([P, OUT], F32, name="bo_bc")
        nc.gpsimd.dma_start(bo_bc[:], bo_d[None, :].to_broadcast((P, OUT)))
        woT_t = []
        for m in range(NM):
            t = woTp.tile([P, OUT], BF16, name=f"woT{m}", tag="woT")
            nc.gpsimd.dma_start(t[:], wo_d[m])
            woT_t.append(t)

        def proj_qk(m):
            """QT_m/KT_m [he_pair=128, s]; evacuated as bf16 in sh-halves."""
            wqm, wkm = wqk[m % 2]
            qkm = []
            evs = []
            for wm, xtiles, nm in ((wqm, qT_t, "qtm"), (wkm, kT_t, "ktm")):
                pst = ps.tile([P, S], F32, tag="ps", name=f"{nm}ps{m}")
                for sh in range(2):
                    for j in range(NK):
                        nc.tensor.matmul(
                            pst[:, sh * 512:(sh + 1) * 512],
                            wm[:, j, :],
                            xtiles[j][:, sh * 512:(sh + 1) * 512],
                            start=(j == 0), stop=(j == NK - 1))
                t = qkp.tile([P, S], BF16, tag="qt", name=f"{nm}{m}")
                qkm.append(t)
                evs.append((t, pst))
            # interleaved half evacuations: scores j=0 only needs the first
            # halves of qtm and ktm
            for sh in range(2):
                for t, pst in evs:
                    nc.vector.tensor_copy(t[:, sh * 512:(sh + 1) * 512],
                                          pst[:, sh * 512:(sh + 1) * 512])
            if m + 2 < NM:
                wqk[m % 2] = load_wqk(m + 2)
            return qkm

        # ---- phase A: Q/K proj for m=0 (starts the PE early)
        qtm_next = proj_qk(0)

        # ---- phase B: V projection (+ ones column)
        v1_t = []
        for i in range(NT):
            pst = ps.tile([P, HE], F32, tag="ps", name=f"vproj{i}")
            for nh in range(2):
                for j in range(NK):
                    nc.tensor.matmul(
                        pst[:, nh * 512:(nh + 1) * 512],
                        vT_t[j][:, i * P:(i + 1) * P],
                        wv_t[j][:, nh * 512:(nh + 1) * 512],
                        start=(j == 0), stop=(j == NK - 1))
            v1 = v1p.tile([P, H, E + 1], BF16, tag="v1", name=f"v1_{i}")
            nc.vector.tensor_copy(v1[:, :, E], ones_h[:])
            nc.vector.tensor_copy(
                v1[:, :, 0:E], pst[:].rearrange("p (h e) -> p h e", e=E))
            v1_t.append(v1)
        ph1.close()

        # ---- phase C: attention m loop
        attm_t = {}

        def normalize_batch(ms):
            """reciprocal of denominators for pairs in ms, broadcast via
            SBUF DMA, normalize att65 -> attm on DVE."""
            b = ms[0] // (NM // 2)
            h0 = 2 * ms[0]
            nh = len(ms) * 2
            nc.vector.reciprocal(recip_f32[b][0:nh, :], sums16[b][0:nh, :])
            nc.vector.tensor_copy(recip_bf[b][0:nh, :], recip_f32[b][0:nh, :])
            nc.sync.dma_start(recip_d[h0:h0 + nh, :], recip_bf[b][0:nh, :])
            for m in ms:
                attm = attp.tile([P, S], BF16, tag="attm", name=f"attm{m}")
                attm_t[m] = attm
                for hh in range(2):
                    rbc = rbcp.tile([E, S], BF16, tag="rbc",
                                    name=f"rbc{m}_{hh}")
                    nc.sync.dma_start(
                        rbc[:], recip_d[2 * m + hh][None, :].to_broadcast((E, S)))
                    nc.vector.tensor_tensor(
                        attm[hh * E:(hh + 1) * E, :],
                        att65[0:E, 2 * m + hh, :], rbc[:], ALU.mult)

        for m in range(NM):
            qtm, ktm = qtm_next

            att_t = {}
            for hh in range(2):
                att_t[hh] = ps_att.tile([E + 1, S], F32, tag="attps",
                                        name=f"att{m}_{hh}")
            for j in range(NT):
                for hh in range(2):
                    hs = slice(hh * E, (hh + 1) * E)
                    sc = ps.tile([P, S], F32, tag="ps", name=f"sc{m}_{j}_{hh}")
                    for sh in range(2):
                        nc.tensor.matmul(
                            sc[:, sh * 512:(sh + 1) * 512],
                            ktm[hs, j * P:(j + 1) * P],
                            qtm[hs, sh * 512:(sh + 1) * 512],
                            start=True, stop=True)
                    # Q/K proj of m+1 slots in right after the last scores:
                    # PE fills the exp latency instead of idling, and the
                    # proj evacuations overlap PV j=7 / scores j=0 of m+1.
                    if j == NT - 1 and hh == 0 and m + 1 < NM:
                        qtm_next = proj_qk(m + 1)
                    ptile = ptp.tile([P, S], BF16, tag="pt",
                                     name=f"p{m}_{j}_{hh}")
                    na = S - EXP_DVE_COLS
                    if na > 0:
                        nc.scalar.activation(ptile[:, 0:na], sc[:, 0:na],
                                             AF.Exp, scale=SCALE)
                    if EXP_DVE_COLS > 0:
                        nc.vector.tensor_scalar(
                            ptile.bitcast(I16)[:, na:S], sc[:, na:S],
                            EXPA, EXPB, ALU.mult, ALU.add)
                    for sh in range(2):
                        nc.tensor.matmul(
                            att_t[hh][:, sh * 512:(sh + 1) * 512],
                            v1_t[j][:, 2 * m + hh, :],
                            ptile[:, sh * 512:(sh + 1) * 512],
                            start=(j == 0), stop=(j == NT - 1))

            # evacuate attendedT + denominator row (unnormalized, bf16)
            for hh in range(2):
                nc.vector.tensor_copy(att65[:, 2 * m + hh, :], att_t[hh][:])
                r = (2 * m + hh) % (H // 2)
                nc.gpsimd.dma_start(
                    sums16[m // (NM // 2)][r:r + 1, :],
                    att65[E:E + 1, 2 * m + hh, :])
            if m == NM // 2 - 1:
                normalize_batch(list(range(NM // 2)))

        normalize_batch(list(range(NM // 2, NM)))

        # ---- phase D: FC
        for st in range(NT):
            for oh in range(2):
                pso = ps_att.tile([P, 512], F32, tag="attps",
                                  name=f"fc{st}_{oh}")
                for m in range(NM):
                    nc.tensor.matmul(
                        pso[:],
                        attm_t[m][:, st * P:(st + 1) * P],
                        woT_t[m][:, oh * 512:(oh + 1) * 512],
                        start=(m == 0), stop=(m == NM - 1))
                ot = outp.tile([P, 512], F32, tag="out", name=f"out{st}_{oh}")
                nc.vector.tensor_tensor(
                    ot[:], pso[:], bo_bc[:, oh * 512:(oh + 1) * 512],
                    ALU.add)
                (nc.sync if (st + oh) % 2 == 0 else nc.gpsimd).dma_start(
                    out_d[st * P:(st + 1) * P, oh * 512:(oh + 1) * 512], ot[:])
    if legalize:
        _legalize_matmul_waits(nc)
    return nc


_NC_CACHE = {}


def _get_nc():
    if "nc" not in _NC_CACHE:
        _NC_CACHE["nc"] = build()
    return _NC_CACHE["nc"]


def _host_pack(query, key, value, Wq, Wk, Wv, Wo, bo):
    """Per-problem host-side layout prep (transpose + cast only)."""
    bf16 = mybir.dt.np(BF16)
    qT = np.ascontiguousarray(query.transpose(0, 2, 1)).astype(bf16)
    kT = np.ascontiguousarray(key.transpose(0, 2, 1)).astype(bf16)
    vT = np.ascontiguousarray(value.transpose(0, 2, 1)).astype(bf16)

    # Wq [h, d, e] with d = ko*128 + ki, h = 2m + hh -> [m, ki, ko, (hh e)]
    def packw(W):
        t = W.reshape(NM, 2, NK, P, E).transpose(0, 3, 2, 1, 4)
        return np.ascontiguousarray(t.reshape(NM, P, NK, P)).astype(bf16)

    wqp = packw(Wq)
    wkp = packw(Wk)
    # Wv [h, d, e] -> [ko, ki, (h e)]
    wvp = np.ascontiguousarray(
        Wv.reshape(H, NK, P, E).transpose(1, 2, 0, 3).reshape(NK, P, HE)
    ).astype(bf16)
    # Wo [out, he] -> [m, he_block, out]
    woT = np.ascontiguousarray(
        Wo.T.reshape(NM, P, OUT)).astype(bf16)
    return qT, kT, vT, wqp, wkp, wvp, woT, bo.astype(np.float32)


def kernel(query, key, value, Wq, Wk, Wv, Wo, bo, **run_kwargs):
    query = np.asarray(query, dtype=np.float32)
    key = np.asarray(key, dtype=np.float32)
    value = np.asarray(value, dtype=np.float32)
    Wq = np.asarray(Wq, dtype=np.float32)
    Wk = np.asarray(Wk, dtype=np.float32)
    Wv = np.asarray(Wv, dtype=np.float32)
    Wo = np.asarray(Wo, dtype=np.float32)
    bo = np.asarray(bo, dtype=np.float32)
    B = query.shape[0]
    assert B == 8, f"expected batch 8, got {B}"

    qT, kT, vT, wqp, wkp, wvp, woT, bo32 = _host_pack(
        query, key, value, Wq, Wk, Wv, Wo, bo)

    nc = _get_nc()
    in_maps = []
    for b in range(B):
        in_maps.append({
            "qT": qT[b], "kT": kT[b], "vT": vT[b],
            "wqp": wqp, "wkp": wkp, "wvp": wvp, "woT": woT, "bo": bo32,
        })
    res = run_bass_kernel_spmd(nc, in_maps, core_ids=list(range(B)),
                               **run_kwargs)
    out = np.stack([r["out"] for r in res.results], axis=0)
    if run_kwargs.get("trace"):
        _NC_CACHE["last_result"] = res
    return out
